# revision 1
# baseline (speedup 1.0000x reference)
"""Causal self-attention with relative position (skew trick), 8-way
head-sharded across Trainium2 NeuronCores.

Shapes (hardcoded): x [4, 2048, 1024], W_attn [1024, 3072], b_attn [3072],
Er [2048, 64], W_proj [1024, 1024], b_proj [1024].  16 heads of 64; each of
the 8 cores handles 2 heads (A, B) for all 4 batches and emits a partial
(pre-reduce) projection output; the host sums the 8 partials (the
tensor-parallel unshard).  b_proj is fed only to core 0 so the sum adds it
exactly once.

Per-core pipeline (per batch):
  1. qkv^T on PE: psum = b + W_c^T x^T  (bias via K=1 ones-matmul init);
     q is copied out pre-scaled by 1/sqrt(64); v is PE-transposed into
     natural [token, hs] layout with an appended ones column (denominator
     trick).
  2. U = scale*(q . Er^T) per head -> DRAM f16 (triangular region only),
     rows of both heads interleaved, row pitch UP=L+512 with pad columns
     preset to -60000 (== causal mask after exp).
  3. S^T tiles [key,query] via packed K=64 matmuls (head A rows 0-63, head
     B rows 64-127 run concurrently in separate PE row-groups); the skewed
     rel-pos tile is read straight from U with a strided *transposing* DMA
     (xbar, f16) and accumulated into the S psum by an identity matmul;
     exp on ACT reads the psum directly.  No max-subtraction: |logits|<~6.
  4. PV with V' stationary: psum[65,512] rows 0-63 = unnormalized y^T,
     row 64 = sum(exp).  Normalize into yn via reciprocal + K=1 broadcast
     matmul.
  5. Partial projection: single K=128 matmul per token block + bias init.

All big matmuls use float32r (1s+8e+11m): 4x faster than fp32 on the PE;
operands are pre-rounded (host inputs) or produced rounded (on-chip tiles).
"""

import numpy as np
from contextlib import ExitStack

import concourse.bass as bass
import concourse.tile as tile
from concourse import mybir
from concourse import bass_utils
from concourse.masks import make_identity

B, L, D = 4, 2048, 1024
NH, HS = 16, 64
NCORES = 8
HPC = 2                 # heads per core
CW = HPC * HS           # 128 head-cols per core
SCALE = 1.0 / 8.0       # 1/sqrt(HS)
F32 = mybir.dt.float32
F32R = mybir.dt.float32r
F16 = mybir.dt.float16
TOKS = B * L
UP = L + 512            # U row pitch; cols [L, UP) = causal-mask pad
NT = L // 128           # token blocks per batch


# walrus in this toolchain rejects instructions carrying >1 sync-wait;
# move excess waits onto preceding same-engine NOPs.
def _split_excess_waits(nc, max_waits=1):
    for f in nc.m.functions:
        for blk in f.blocks:
            new_insts = []
            for inst in blk.instructions:
                si = getattr(inst, "sync_info", None)
                if si is not None and si.on_wait and len(si.on_wait) > max_waits:
                    waits = list(si.on_wait)
                    chunks = [waits[i:i + max_waits]
                              for i in range(0, len(waits), max_waits)]
                    for j, ch in enumerate(chunks[:-1]):
                        new_insts.append(mybir.InstNoOp(
                            name=f"{inst.name}-waitsplit{j}",
                            engine=inst.engine,
                            sync_info=mybir.SyncInfo(on_wait=ch, on_update=[]),
                            bass_nofuse=True,
                        ))
                    si.on_wait = chunks[-1]
                new_insts.append(inst)
            blk.instructions[:] = new_insts


def jb_min(ib):
    # U row-block ib (128 rows at i0=128*ib) needs Er-index columns
    # j >= 2047 - (i0+127); 512-wide column blocks from jb_min(ib) to 3.
    return max(0, (1920 - 128 * ib) // 512)


def build_program(phases=("qkv", "u", "attn", "proj")):
    nc = bass.Bass("TRN2", target_bir_lowering=False, debug=False,
                   num_devices=NCORES)
    xT = nc.declare_dram_parameter("xT", [D, TOKS], F32R, isOutput=False)
    wqkv = nc.declare_dram_parameter("wqkv", [D, 3 * CW], F32R, isOutput=False)
    bqkv = nc.declare_dram_parameter("bqkv", [3 * CW], F32R, isOutput=False)
    ertd = nc.declare_dram_parameter("ertd", [2 * HS, L], F32R, isOutput=False)
    wp = nc.declare_dram_parameter("wp", [CW, D], F32R, isOutput=False)
    bp = nc.declare_dram_parameter("bp", [D], F32R, isOutput=False)
    part = nc.declare_dram_parameter("part", [TOKS, D], F32, isOutput=True)
    # U scratch: [slot(batch%2), i, head, col]; pitch UP, pad cols hold the
    # causal mask.  Interleaving heads lets one DMA write both heads' tiles.
    u_scr = nc.dram_tensor("u_scr", [2, L, 2, UP], F16)

    do = lambda p: p in phases
    with tile.TileContext(nc) as tc, ExitStack() as ctx, \
            nc.allow_low_precision(reason="f32r matmul operands; fp32 psum accum"):
        singles = ctx.enter_context(tc.tile_pool(name="singles", bufs=1))
        pb = ctx.enter_context(tc.tile_pool(name="perbatch", bufs=2))
        xin = ctx.enter_context(tc.tile_pool(name="xin", bufs=4))
        work = ctx.enter_context(tc.tile_pool(name="work", bufs=3))
        workb = ctx.enter_context(tc.tile_pool(name="workb", bufs=6))
        psp = ctx.enter_context(tc.tile_pool(name="psp", bufs=6, space="PSUM"))
        ps_y = ctx.enter_context(tc.tile_pool(name="ps_y", bufs=2, space="PSUM"))

        # ---- constants / weights ----
        w_sb = singles.tile([128, 8 * 3 * CW], F32R)
        nc.sync.dma_start(w_sb[:], wqkv.ap().rearrange(
            "(kb p) m -> kb p m", p=128).transpose([1, 0, 2]))
        bq_row = singles.tile([1, 3 * CW], F32R)
        nc.sync.dma_start(bq_row[:], bqkv.ap().unsqueeze(0))
        ertd_sb = singles.tile([128, L], F32R)
        nc.sync.dma_start(ertd_sb[:], ertd.ap())
        wp_sb = singles.tile([CW, D], F32R)
        nc.sync.dma_start(wp_sb[:], wp.ap())
        bp_row = singles.tile([1, D], F32R)
        nc.sync.dma_start(bp_row[:], bp.ap().unsqueeze(0))
        onesf = singles.tile([128, 512], F32)
        nc.vector.memset(onesf[:], 1.0)
        ones_row = singles.tile([1, 512], F32R)
        nc.vector.tensor_copy(ones_row[:], onesf[0:1, :])
        ident = singles.tile([128, 128], F32)
        make_identity(nc, ident[:])
        ident16 = singles.tile([128, 128], F16)
        make_identity(nc, ident16[:])
        # preset U pad columns to a large negative logit (exp -> 0); finite,
        # not -inf, because the identity inject multiplies pad values by 0.
        bias_sb = singles.tile([128, D], F32)
        for nb in range(2):
            pbi = psp.tile([128, 512], F32, tag="p")
            nc.tensor.matmul(pbi[:], ones_row[0:1, 0:128],
                             bp_row[0:1, nb * 512:(nb + 1) * 512],
                             start=True, stop=True)
            nc.vector.tensor_copy(bias_sb[:, nb * 512:(nb + 1) * 512], pbi[:])
        padf = singles.tile([128, 2 * 512], F16)
        nc.vector.memset(padf[:], -60000.0)
        for slot in range(2):
            for rg in range(16):
                nc.gpsimd.dma_start(
                    u_scr.ap()[slot, rg * 128:(rg + 1) * 128, :, L:UP], padf[:])

        def emit_build(b):
            """qkv + U emission closures for batch b (allocates its tiles)."""
            st = dict(slot=b % 2)
            st["qT"] = pb.tile([128, L], F32R, tag="qT", name="qT")
            st["kT"] = pb.tile([128, L], F32R, tag="kT", name="kT")
            st["va"] = pb.tile([128, NT * (HS + 1)], F32R, tag="va", name="va")
            st["vb"] = pb.tile([128, NT * (HS + 1)], F32R, tag="vb", name="vb")
            parts = []

            def ones_cols(st=st):
                for vt in (st["va"], st["vb"]):
                    ocol = bass.AP(vt[:].tensor, vt[:].offset + HS,
                                   [vt[:].ap[0], [HS + 1, NT], [1, 1]])
                    nc.vector.tensor_copy(ocol, onesf[:, 0:NT].unsqueeze(2))
            parts.append(ones_cols)

            def qkv_chunk(tch, b=b, st=st):
                qT, kT, va, vb = st["qT"], st["kT"], st["va"], st["vb"]
                col0 = b * L + tch * 512
                halves = []
                for hf in range(2):     # finer DMA granularity: 4 kb each
                    xc = xin.tile([128, 4 * 512], F32R, tag="xchunk", name="xc")
                    nc.sync.dma_start(
                        xc[:],
                        xT.ap()[4 * hf * 128:(4 * hf + 4) * 128,
                                col0:col0 + 512].rearrange(
                            "(kb p) n -> kb p n", p=128).transpose([1, 0, 2]))
                    halves.append(xc)
                for m in range(3):      # q, k, v col-groups of 128
                    ps = psp.tile([128, 512], F32, tag="p")
                    nc.tensor.matmul(ps[:], bq_row[0:1, m * 128:(m + 1) * 128],
                                     ones_row[0:1, :], start=True, stop=False)
                    for kb in range(8):
                        nc.tensor.matmul(
                            ps[:],
                            w_sb[:, kb * 384 + m * 128: kb * 384 + (m + 1) * 128],
                            halves[kb // 4][:, (kb % 4) * 512:(kb % 4 + 1) * 512],
                            start=False, stop=(kb == 7))
                    if m == 0:
                        nc.scalar.activation(qT[:, tch * 512:(tch + 1) * 512], ps[:],
                                             mybir.ActivationFunctionType.Copy,
                                             scale=SCALE)
                    elif m == 1:
                        nc.scalar.activation(kT[:, tch * 512:(tch + 1) * 512], ps[:],
                                             mybir.ActivationFunctionType.Copy)
                    else:
                        vtmp = work.tile([128, 512], F32, tag="vtmp")
                        nc.vector.tensor_copy(vtmp[:], ps[:])
                        for s in range(4):
                            tk = tch * 4 + s
                            pt = psp.tile([128, 512], F32, tag="p")
                            nc.tensor.transpose(pt[:, 0:128],
                                                vtmp[:, s * 128:(s + 1) * 128],
                                                ident[:])
                            nc.vector.tensor_copy(
                                va[:, tk * (HS + 1): tk * (HS + 1) + HS],
                                pt[:, 0:HS])
                            nc.vector.tensor_copy(
                                vb[:, tk * (HS + 1): tk * (HS + 1) + HS],
                                pt[:, HS:2 * HS])

            def u_block(ib, st=st):
                qT, slot = st["qT"], st["slot"]
                i0 = ib * 128
                jbs = list(range(jb_min(ib), 4))
                groups = [jbs[:1], jbs[1:]] if len(jbs) % 2 else                          [jbs[:2], jbs[2:]]
                for grp in groups:
                    if not grp:
                        continue
                    w = 512 * len(grp)
                    ucmb = workb.tile([128, 2 * 1024], F16, tag="ubf")
                    for gi, jb in enumerate(grp):
                        pua = psp.tile([128, 512], F32, tag="p")
                        pub = psp.tile([128, 512], F32, tag="p")
                        nc.tensor.matmul(pua[:], qT[0:HS, i0:i0 + 128],
                                         ertd_sb[0:HS, jb * 512:(jb + 1) * 512],
                                         start=True, stop=True)
                        nc.tensor.matmul(pub[:], qT[HS:128, i0:i0 + 128],
                                         ertd_sb[HS:128, jb * 512:(jb + 1) * 512],
                                         start=True, stop=True)
                        nc.vector.tensor_copy(
                            ucmb[:, gi * 512: gi * 512 + 512], pua[:])
                        nc.scalar.activation(
                            ucmb[:, w + gi * 512: w + gi * 512 + 512], pub[:],
                            mybir.ActivationFunctionType.Copy)
                    nc.sync.dma_start(
                        u_scr.ap()[slot, i0:i0 + 128, :,
                                   grp[0] * 512: grp[0] * 512 + w],
                        ucmb[:, 0: 2 * w])

            if do("qkv"):
                for tch in range(4):
                    parts.append(lambda tch=tch: qkv_chunk(tch))
                if do("u"):
                    for tch in range(4):
                        parts.append(lambda tch=tch: [u_block(4 * tch + j)
                                                      for j in range(4)])
            return st, parts

        def emit_attn(b, st):
            """attention + projection closures for batch b."""
            qT, kT, va, vb, slot = (st["qT"], st["kT"], st["va"], st["vb"],
                                    st["slot"])
            yn = pb.tile([128, L], F32R, tag="yn")
            parts = []

            def attn_ib(ib5):
                i0 = ib5 * 512
                pyA = ps_y.tile([HS + 1, 512], F32, tag="y")
                pyB = ps_y.tile([HS + 1, 512], F32, tag="y")
                n_mb = 4 * (ib5 + 1)
                for mb in range(n_mb):
                    m0 = mb * 128
                    sss, srels = [], []
                    for h in range(2):
                        ss = psp.tile([128, 512], F32, tag="p")
                        nc.tensor.matmul(
                            ss[:], kT[h * HS:(h + 1) * HS, m0:m0 + 128],
                            qT[h * HS:(h + 1) * HS, i0:i0 + 512],
                            start=True, stop=False)
                        base_h = slot * (L * 2 * UP) + h * UP
                        srel = workb.tile([128, 512], F16, tag="srel")
                        nc.sync.dma_start_transpose(
                            srel[:],
                            bass.AP(u_scr, base_h + (L - 1)
                                    + i0 * (2 * UP - 1) + m0,
                                    [[2 * UP - 1, 512], [1, 128]]))
                        sss.append(ss)
                        srels.append(srel)
                    for ss, srel, py, vt in zip(sss, srels, (pyA, pyB),
                                                (va, vb)):
                        nc.tensor.matmul(ss[:], ident16[:], srel[:],
                                         start=False, stop=True)
                        et = work.tile([128, 512], F32R, tag="et")
                        nc.scalar.activation(et[:], ss[:],
                                             mybir.ActivationFunctionType.Exp)
                        nc.tensor.matmul(
                            py[:], vt[:, mb * (HS + 1):(mb + 1) * (HS + 1)],
                            et[:], start=(mb == 0), stop=(mb == n_mb - 1))
                for h, py in enumerate((pyA, pyB)):
                    recip = work.tile([1, 512], F32R, tag="recip")
                    nc.vector.reciprocal(recip[:], py[HS:HS + 1, :])
                    pbc = psp.tile([128, 512], F32, tag="p")
                    nc.tensor.matmul(pbc[0:HS, :], ones_row[0:1, 0:HS],
                                     recip[:], start=True, stop=True)
                    bc_sb = work.tile([HS, 512], F32, tag="bcsb")
                    nc.scalar.activation(bc_sb[:], pbc[0:HS, :],
                                         mybir.ActivationFunctionType.Copy)
                    nc.vector.tensor_mul(yn[h * HS:(h + 1) * HS, i0:i0 + 512],
                                         py[0:HS, :], bc_sb[:])

            def proj_blk(tkg, b=b):
                for tk in range(4 * tkg, 4 * tkg + 4):
                    t0 = tk * 128
                    for nb in range(2):
                        po = psp.tile([128, 512], F32, tag="p")
                        nc.tensor.matmul(po[:], yn[:, t0:t0 + 128],
                                         wp_sb[:, nb * 512:(nb + 1) * 512],
                                         start=True, stop=True)
                        osb = work.tile([128, 512], F32, tag="osb")
                        nc.vector.tensor_add(osb[:], po[:],
                                             bias_sb[:, nb * 512:(nb + 1) * 512])
                        nc.gpsimd.dma_start(
                            part.ap()[b * L + t0: b * L + t0 + 128,
                                      nb * 512:(nb + 1) * 512], osb[:])

            if do("attn"):
                for ib5 in range(4):
                    parts.append(lambda ib5=ib5: attn_ib(ib5))
                if do("proj"):
                    for tkg in range(4):
                        parts.append(lambda tkg=tkg: proj_blk(tkg))
            elif do("proj"):
                for tkg in range(4):
                    parts.append(lambda tkg=tkg: proj_blk(tkg))
            return parts

        # software-pipelined emission: batch b's attention/projection is
        # interleaved with batch b+1's qkv/U so the scheduler can overlap
        # them across engines.
        def interleave(xs, ys):
            out, i, j = [], 0, 0
            while i < len(xs) or j < len(ys):
                if i < len(xs):
                    out.append(xs[i]); i += 1
                if j < len(ys):
                    out.append(ys[j]); j += 1
            return out

        st, build = emit_build(0)
        for p in build:
            p()
        for b in range(B):
            consume = emit_attn(b, st)
            if b + 1 < B:
                st, build = emit_build(b + 1)
            else:
                build = []
            for p in consume + build:
                p()

    return nc


def _round_f32r(a):
    """Round fp32 to fp32r (round-to-nearest-even to 11 mantissa bits) —
    the matmul engine requires f32r operands pre-rounded."""
    b = np.ascontiguousarray(a, np.float32).view(np.uint32)
    r = (b + np.uint32(0x7FF) + ((b >> np.uint32(12)) & np.uint32(1))) \
        & np.uint32(0xFFFFF000)
    return r.view(np.float32)


def make_in_maps(x, W_attn, b_attn, Er, W_proj, b_proj):
    x = np.asarray(x, np.float32)
    W_attn = np.asarray(W_attn, np.float32)
    b_attn = np.asarray(b_attn, np.float32)
    Er = np.asarray(Er, np.float32)
    W_proj = np.asarray(W_proj, np.float32)
    b_proj = np.asarray(b_proj, np.float32)
    xT = _round_f32r(np.ascontiguousarray(x.reshape(TOKS, D).T))
    ErT = np.ascontiguousarray(Er.T)
    ertd = _round_f32r(np.concatenate([ErT, ErT], axis=0))
    zeros_bp = np.zeros_like(b_proj)
    in_maps = []
    for c in range(NCORES):
        q0 = CW * c
        wq = W_attn[:, q0:q0 + CW]
        wk = W_attn[:, D + q0:D + q0 + CW]
        wv = W_attn[:, 2 * D + q0:2 * D + q0 + CW]
        in_maps.append(dict(
            xT=xT,
            wqkv=_round_f32r(np.concatenate([wq, wk, wv], axis=1)),
            bqkv=_round_f32r(np.concatenate(
                [b_attn[q0:q0 + CW], b_attn[D + q0:D + q0 + CW],
                 b_attn[2 * D + q0:2 * D + q0 + CW]])),
            ertd=ertd,
            wp=_round_f32r(W_proj[q0:q0 + CW, :]),
            bp=_round_f32r(b_proj if c == 0 else zeros_bp),
        ))
    return in_maps


_cached_nc = None


def kernel(x, W_attn, b_attn, Er, W_proj, b_proj):
    global _cached_nc
    if _cached_nc is None:
        _cached_nc = build_program()
        _split_excess_waits(_cached_nc)
    nc = _cached_nc
    in_maps = make_in_maps(x, W_attn, b_attn, Er, W_proj, b_proj)
    res = bass_utils.run_bass_kernel_spmd(nc, in_maps, list(range(NCORES)))
    out = np.zeros((TOKS, D), np.float32)
    for c in range(NCORES):
        out += res.results[c]["part"]
    return out.reshape(B, L, D)



# revision 26
# speedup vs baseline: 97.5839x; 97.5839x over previous
"""Causal self-attention with relative position (skew trick), 8-way
head-sharded across Trainium2 NeuronCores.

Shapes (hardcoded): x [4, 2048, 1024], W_attn [1024, 3072], b_attn [3072],
Er [2048, 64], W_proj [1024, 1024], b_proj [1024].  16 heads of 64; each of
the 8 cores handles 2 heads (A, B) for all 4 batches and emits a partial
(pre-reduce) projection output; the host sums the 8 partials (the
tensor-parallel unshard).  b_proj is fed only to core 0 so the sum adds it
exactly once.

Per-core pipeline (per batch):
  1. qkv^T on PE: psum = b + W_c^T x^T  (bias via K=1 ones-matmul init);
     q is copied out pre-scaled by 1/sqrt(64); v is PE-transposed into
     natural [token, hs] layout with an appended ones column (denominator
     trick).
  2. U = scale*(q . Er^T) per head -> DRAM f16 (triangular region only),
     rows of both heads interleaved, row pitch UP=L+512 with pad columns
     preset to -60000 (== causal mask after exp).
  3. S^T tiles [key,query] via packed K=64 matmuls (head A rows 0-63, head
     B rows 64-127 run concurrently in separate PE row-groups); the skewed
     rel-pos tile is read straight from U with a strided *transposing* DMA
     (xbar, f16) and accumulated into the S psum by an identity matmul;
     exp on ACT reads the psum directly.  No max-subtraction: |logits|<~6.
  4. PV with V' stationary: psum[65,512] rows 0-63 = unnormalized y^T,
     row 64 = sum(exp).  Normalize into yn via reciprocal + K=1 broadcast
     matmul.
  5. Partial projection: single K=128 matmul per token block + bias init.

All big matmuls use float32r (1s+8e+11m): 4x faster than fp32 on the PE;
operands are pre-rounded (host inputs) or produced rounded (on-chip tiles).
"""

import numpy as np
from contextlib import ExitStack

import concourse.bass as bass
import concourse.tile as tile
from concourse import mybir
from concourse import bass_utils
from concourse.masks import make_identity
from concourse import library_config

B, L, D = 4, 2048, 1024
NH, HS = 16, 64
NCORES = 8
HPC = 2                 # heads per core
CW = HPC * HS           # 128 head-cols per core
SCALE = 1.0 / 8.0       # 1/sqrt(HS)
F32 = mybir.dt.float32
F32R = mybir.dt.float32r
F16 = mybir.dt.float16
BF16 = mybir.dt.bfloat16
TOKS = B * L
UP = L + 512            # U row pitch; cols [L, UP) = causal-mask pad
NT = L // 128           # token blocks per batch


# walrus in this toolchain rejects instructions carrying >1 sync-wait;
# move excess waits onto preceding same-engine NOPs.
def _split_excess_waits(nc, max_waits=1):
    for f in nc.m.functions:
        for blk in f.blocks:
            new_insts = []
            for inst in blk.instructions:
                si = getattr(inst, "sync_info", None)
                if si is not None and si.on_wait and len(si.on_wait) > max_waits:
                    waits = list(si.on_wait)
                    chunks = [waits[i:i + max_waits]
                              for i in range(0, len(waits), max_waits)]
                    for j, ch in enumerate(chunks[:-1]):
                        new_insts.append(mybir.InstNoOp(
                            name=f"{inst.name}-waitsplit{j}",
                            engine=inst.engine,
                            sync_info=mybir.SyncInfo(on_wait=ch, on_update=[]),
                            bass_nofuse=True,
                        ))
                    si.on_wait = chunks[-1]
                new_insts.append(inst)
            blk.instructions[:] = new_insts


def jb_min(ib):
    # U row-block ib (128 rows at i0=128*ib) needs Er-index columns
    # j >= 2047 - (i0+127); 512-wide column blocks from jb_min(ib) to 3.
    return max(0, (1920 - 128 * ib) // 512)


def build_program(phases=("qkv", "u", "attn", "proj"), reps=1):
    nc = bass.Bass("TRN2", target_bir_lowering=False, debug=False,
                   num_devices=NCORES)
    xT = nc.declare_dram_parameter("xT", [D, TOKS], BF16, isOutput=False)
    wqkv = nc.declare_dram_parameter("wqkv", [D, 3 * CW], BF16, isOutput=False)
    bqkv = nc.declare_dram_parameter("bqkv", [3 * CW], F32, isOutput=False)
    ertd = nc.declare_dram_parameter("ertd", [2 * HS, L], F32R, isOutput=False)
    wp = nc.declare_dram_parameter("wp", [CW, D], F32R, isOutput=False)
    part = nc.declare_dram_parameter("part", [TOKS, D], F16, isOutput=True)
    # U scratch: [slot(batch%2), i, head, col]; pitch UP, pad cols hold the
    # causal mask.  Interleaving heads lets one DMA write both heads' tiles.
    u_scr = nc.dram_tensor("u_scr", [2, L, 2, UP], F16)

    do = lambda p: p in phases
    with tile.TileContext(nc) as tc, ExitStack() as ctx, \
            nc.allow_low_precision(reason="f32r matmul operands; fp32 psum accum"):
        singles = ctx.enter_context(tc.tile_pool(name="singles", bufs=1))
        pb = ctx.enter_context(tc.tile_pool(name="perbatch", bufs=2))
        xin = ctx.enter_context(tc.tile_pool(name="xin", bufs=4))
        work = ctx.enter_context(tc.tile_pool(name="work", bufs=3))
        etp = ctx.enter_context(tc.tile_pool(name="etp", bufs=6))
        srlp = ctx.enter_context(tc.tile_pool(name="srlp", bufs=12))
        ucp = ctx.enter_context(tc.tile_pool(name="ucp", bufs=6))
        psp = ctx.enter_context(tc.tile_pool(name="psp", bufs=6, space="PSUM"))
        ps_y = ctx.enter_context(tc.tile_pool(name="ps_y", bufs=2, space="PSUM"))
        ps_bld = psp
        ps_att = psp

        # ---- constants / weights ----
        w_sb = singles.tile([128, 8 * 3 * CW], BF16)
        for kb in range(8):
            nc.sync.dma_start(w_sb[:, kb * 384:(kb + 1) * 384],
                              wqkv.ap()[kb * 128:(kb + 1) * 128, :])
        # qkv bias as per-partition columns: bq_cols[p, g] = bqkv[g*128+p]
        # (g: 0=q pre-scaled on host, 1=k, 2=v); applied in the psum->SBUF
        # copies via DVE tensor_scalar, so no bias-init matmuls are needed.
        bq_cols = singles.tile([128, 3], F32)
        nc.sync.dma_start(bq_cols[:], bqkv.ap().rearrange("(g p) -> p g", p=128))
        ertd_sb = singles.tile([128, L], F32R)
        nc.sync.dma_start(ertd_sb[:], ertd.ap())
        wp_sb = singles.tile([CW, D], F32R)
        nc.sync.dma_start(wp_sb[:], wp.ap())
        onesf = singles.tile([128, 512], F32)
        nc.vector.memset(onesf[:], 1.0)
        ones_row = singles.tile([1, 512], F32R)
        nc.vector.tensor_copy(ones_row[:], onesf[0:1, :])
        ident = singles.tile([128, 128], F32)
        make_identity(nc, ident[:])
        ident16 = singles.tile([128, 128], F16)
        make_identity(nc, ident16[:])
        # preset U pad columns to a large negative logit (exp -> 0); finite,
        # not -inf, because the identity inject multiplies pad values by 0.
        padf = singles.tile([128, 2 * 512], F16)
        nc.vector.memset(padf[:], -60000.0)
        for slot in range(2):
            for rg in range(16):
                nc.gpsimd.dma_start(
                    u_scr.ap()[slot, rg * 128:(rg + 1) * 128, :, L:UP], padf[:])

        def emit_build(b):
            """qkv + U emission closures for batch b (allocates its tiles)."""
            st = dict(slot=b % 2)
            st["qT"] = pb.tile([128, L], F32R, tag="qT", name="qT")
            st["kT"] = pb.tile([128, L], F32R, tag="kT", name="kT")
            st["va"] = pb.tile([128, NT * (HS + 1)], F32R, tag="va", name="va")
            st["vb"] = pb.tile([128, NT * (HS + 1)], F32R, tag="vb", name="vb")
            parts = []

            def ones_cols(st=st):
                for vt in (st["va"], st["vb"]):
                    ocol = bass.AP(vt[:].tensor, vt[:].offset + HS,
                                   [vt[:].ap[0], [HS + 1, NT], [1, 1]])
                    nc.vector.tensor_copy(ocol, onesf[:, 0:NT].unsqueeze(2))
            parts.append(ones_cols)

            def qkv_chunk(tch, b=b, st=st):
                qT, kT, va, vb = st["qT"], st["kT"], st["va"], st["vb"]
                col0 = b * L + tch * 512
                xc = xin.tile([128, 8 * 512], BF16, tag="xchunk", name="xc")
                nc.sync.dma_start(
                    xc[:],
                    xT.ap()[:, col0:col0 + 512].rearrange(
                        "(kb p) n -> kb p n", p=128).transpose([1, 0, 2]))
                for m in range(3):      # q, k, v col-groups of 128
                    ps = ps_bld.tile([128, 512], F32, tag="p")
                    for kb in range(8):
                        nc.tensor.matmul(
                            ps[:],
                            w_sb[:, kb * 384 + m * 128: kb * 384 + (m + 1) * 128],
                            xc[:, kb * 512:(kb + 1) * 512],
                            start=(kb == 0), stop=(kb == 7))
                    if m == 0:
                        # qT = ps*SCALE + b_q*SCALE (host pre-scales the q bias)
                        nc.vector.tensor_scalar(
                            qT[:, tch * 512:(tch + 1) * 512], ps[:],
                            SCALE, bq_cols[:, 0:1],
                            mybir.AluOpType.mult, mybir.AluOpType.add)
                    elif m == 1:
                        nc.vector.tensor_scalar_add(
                            kT[:, tch * 512:(tch + 1) * 512], ps[:],
                            bq_cols[:, 1:2])
                    else:
                        vtmp = work.tile([128, 512], F32, tag="vtmp")
                        nc.vector.tensor_scalar_add(vtmp[:], ps[:],
                                                    bq_cols[:, 2:3])
                        for s in range(4):
                            tk = tch * 4 + s
                            pt = ps_bld.tile([128, 512], F32, tag="p")
                            nc.tensor.transpose(pt[:, 0:128],
                                                vtmp[:, s * 128:(s + 1) * 128],
                                                ident[:])
                            nc.vector.tensor_copy(
                                va[:, tk * (HS + 1): tk * (HS + 1) + HS],
                                pt[:, 0:HS])
                            nc.vector.tensor_copy(
                                vb[:, tk * (HS + 1): tk * (HS + 1) + HS],
                                pt[:, HS:2 * HS])

            def u_block(ib, st=st):
                qT, slot = st["qT"], st["slot"]
                i0 = ib * 128
                jbs = list(range(jb_min(ib), 4))
                groups = [jbs[:1], jbs[1:]] if len(jbs) % 2 else                          [jbs[:2], jbs[2:]]
                for grp in groups:
                    if not grp:
                        continue
                    w = 512 * len(grp)
                    ucmb = ucp.tile([128, 2 * 1024], F16, tag="ubf")
                    for gi, jb in enumerate(grp):
                        pua = ps_bld.tile([128, 512], F32, tag="p")
                        pub = ps_bld.tile([128, 512], F32, tag="p")
                        nc.tensor.matmul(pua[:], qT[0:HS, i0:i0 + 128],
                                         ertd_sb[0:HS, jb * 512:(jb + 1) * 512],
                                         start=True, stop=True)
                        nc.tensor.matmul(pub[:], qT[HS:128, i0:i0 + 128],
                                         ertd_sb[HS:128, jb * 512:(jb + 1) * 512],
                                         start=True, stop=True)
                        nc.vector.tensor_copy(
                            ucmb[:, gi * 512: gi * 512 + 512], pua[:])
                        nc.scalar.activation(
                            ucmb[:, w + gi * 512: w + gi * 512 + 512], pub[:],
                            mybir.ActivationFunctionType.Copy)
                    nc.sync.dma_start(
                        u_scr.ap()[slot, i0:i0 + 128, :,
                                   grp[0] * 512: grp[0] * 512 + w],
                        ucmb[:, 0: 2 * w])

            if do("qkv"):
                for tch in range(4):
                    parts.append(lambda tch=tch: qkv_chunk(tch))
                if do("u"):
                    for tch in range(4):
                        parts.append(lambda tch=tch: [u_block(4 * tch + j)
                                                      for j in range(4)])
            return st, parts

        def emit_attn(b, st):
            """attention + projection closures for batch b."""
            qT, kT, va, vb, slot = (st["qT"], st["kT"], st["va"], st["vb"],
                                    st["slot"])
            yn = pb.tile([128, L], F32R, tag="yn")
            parts = []

            def attn_ib(ib5):
                i0 = ib5 * 512
                pyA = ps_y.tile([HS + 1, 512], F32, tag="y")
                pyB = ps_y.tile([HS + 1, 512], F32, tag="y")
                n_mb = 4 * (ib5 + 1)
                for mb in range(n_mb):
                    m0 = mb * 128
                    sss, srels = [], []
                    for h in range(2):
                        ss = ps_att.tile([128, 512], F32, tag="p")
                        nc.tensor.matmul(
                            ss[:], kT[h * HS:(h + 1) * HS, m0:m0 + 128],
                            qT[h * HS:(h + 1) * HS, i0:i0 + 512],
                            start=True, stop=False)
                        base_h = slot * (L * 2 * UP) + h * UP
                        srel = srlp.tile([128, 512], F16, tag="srel")
                        nc.sync.dma_start_transpose(
                            srel[:],
                            bass.AP(u_scr, base_h + (L - 1)
                                    + i0 * (2 * UP - 1) + m0,
                                    [[2 * UP - 1, 512], [1, 128]]))
                        sss.append(ss)
                        srels.append(srel)
                    for ss, srel, py, vt in zip(sss, srels, (pyA, pyB),
                                                (va, vb)):
                        nc.tensor.matmul(ss[:], ident16[:], srel[:],
                                         start=False, stop=True)
                        et = etp.tile([128, 512], F32R, tag="et")
                        nc.scalar.activation(et[:], ss[:],
                                             mybir.ActivationFunctionType.Exp)
                        nc.tensor.matmul(
                            py[:], vt[:, mb * (HS + 1):(mb + 1) * (HS + 1)],
                            et[:], start=(mb == 0), stop=(mb == n_mb - 1))
                for h, py in enumerate((pyA, pyB)):
                    recip = work.tile([1, 512], F32R, tag="recip")
                    nc.vector.reciprocal(recip[:], py[HS:HS + 1, :])
                    pbc = ps_att.tile([128, 512], F32, tag="p")
                    nc.tensor.matmul(pbc[0:HS, :], ones_row[0:1, 0:HS],
                                     recip[:], start=True, stop=True)
                    bc_sb = work.tile([HS, 512], F32, tag="bcsb")
                    nc.scalar.activation(bc_sb[:], pbc[0:HS, :],
                                         mybir.ActivationFunctionType.Copy)
                    nc.vector.tensor_mul(yn[h * HS:(h + 1) * HS, i0:i0 + 512],
                                         py[0:HS, :], bc_sb[:])

            def proj_blk(tkg, b=b):
                for tk in range(4 * tkg, 4 * tkg + 4):
                    t0 = tk * 128
                    osb = work.tile([128, 2 * 512], F16, tag="osb")
                    for nb in range(2):
                        po = ps_att.tile([128, 512], F32, tag="p")
                        nc.tensor.matmul(po[:], yn[:, t0:t0 + 128],
                                         wp_sb[:, nb * 512:(nb + 1) * 512],
                                         start=True, stop=True)
                        nc.vector.tensor_copy(
                            osb[:, nb * 512:(nb + 1) * 512], po[:])
                    nc.sync.dma_start(
                        part.ap()[b * L + t0: b * L + t0 + 128, :], osb[:])

            if do("attn"):
                for ib5 in range(4):
                    parts.append(lambda ib5=ib5: attn_ib(ib5))
                if do("proj"):
                    for tkg in range(4):
                        parts.append(lambda tkg=tkg: proj_blk(tkg))
            elif do("proj"):
                for tkg in range(4):
                    parts.append(lambda tkg=tkg: proj_blk(tkg))
            return parts

        # software-pipelined emission: batch b's attention/projection is
        # interleaved with batch b+1's qkv/U so the scheduler can overlap
        # them across engines.
        def emit_all():
            st, build = emit_build(0)
            for p in build:
                p()
            for b in range(B):
                consume = emit_attn(b, st)
                if b + 1 < B:
                    st, build = emit_build(b + 1)
                else:
                    build = []
                for p in consume + build:
                    p()

        if reps > 1:
            # hardware loop over the whole body: used only by the timing
            # harness (T(reps)-T(1) isolates per-iteration device time from
            # the ~50-100ms axon dispatch overhead)
            with tc.For_i(0, reps):
                emit_all()
        else:
            emit_all()

    return nc


def _round_f32r(a):
    """Round fp32 to fp32r (round-to-nearest-even to 11 mantissa bits) —
    the matmul engine requires f32r operands pre-rounded."""
    b = np.ascontiguousarray(a, np.float32).view(np.uint32)
    r = (b + np.uint32(0x7FF) + ((b >> np.uint32(12)) & np.uint32(1))) \
        & np.uint32(0xFFFFF000)
    return r.view(np.float32)


def make_in_maps(x, W_attn, b_attn, Er, W_proj, b_proj):
    import ml_dtypes
    bf16 = ml_dtypes.bfloat16
    x = np.asarray(x, np.float32)
    W_attn = np.asarray(W_attn, np.float32)
    b_attn = np.asarray(b_attn, np.float32)
    Er = np.asarray(Er, np.float32)
    W_proj = np.asarray(W_proj, np.float32)
    xT = np.ascontiguousarray(x.reshape(TOKS, D).T).astype(bf16)
    ErT = np.ascontiguousarray(Er.T)
    ertd = _round_f32r(np.concatenate([ErT, ErT], axis=0))
    in_maps = []
    for c in range(NCORES):
        q0 = CW * c
        wq = W_attn[:, q0:q0 + CW]
        wk = W_attn[:, D + q0:D + q0 + CW]
        wv = W_attn[:, 2 * D + q0:2 * D + q0 + CW]
        in_maps.append(dict(
            xT=xT,
            wqkv=np.ascontiguousarray(
                np.concatenate([wq, wk, wv], axis=1)).astype(bf16),
            bqkv=np.concatenate(
                [b_attn[q0:q0 + CW] * SCALE, b_attn[D + q0:D + q0 + CW],
                 b_attn[2 * D + q0:2 * D + q0 + CW]]).astype(np.float32),
            ertd=ertd,
            wp=_round_f32r(W_proj[q0:q0 + CW, :]),
        ))
    return in_maps


_cached_nc = None


def kernel(x, W_attn, b_attn, Er, W_proj, b_proj):
    global _cached_nc
    if _cached_nc is None:
        _cached_nc = build_program()
        _split_excess_waits(_cached_nc)
    nc = _cached_nc
    in_maps = make_in_maps(x, W_attn, b_attn, Er, W_proj, b_proj)
    res = bass_utils.run_bass_kernel_spmd(nc, in_maps, list(range(NCORES)))
    out = np.zeros((TOKS, D), np.float32)
    for c in range(NCORES):
        out += res.results[c]["part"].astype(np.float32)
    out += np.asarray(b_proj, np.float32)[None, :]
    return out.reshape(B, L, D)

